# revision 1
# baseline (speedup 1.0000x reference)
"""Trainium2 Bass kernel for nn_Decoder (GNN message passing):
LSTM(1 step) -> GCNConv -> ReLU -> GCNConv -> Linear -> ReLU on a
100K-node / 1.6M-edge graph, SPMD across 8 NeuronCores.

Strategy (dst-node sharding):
- Core c owns nodes [c*12500, (c+1)*12500) and all edges into them.
- Per-node compute (LSTM, x@W transforms) runs feature-major [128, nodes]
  so all matmuls need zero transposes and biases are per-partition.
- The GCN propagate gathers transformed rows from a bf16 node-major table
  in DRAM (built via AllGather of the 8 shards) with gpsimd.dma_gather,
  then scatter-adds via PE matmul with an on-chip selection matrix
  (tensor_scalar: iota==dst_idx -> * norm), accumulated in PSUM per
  128-dst block.
"""

from contextlib import ExitStack

import numpy as np
import ml_dtypes

import concourse.bacc as bacc
import concourse.mybir as mybir
import concourse.tile as tile
from concourse.bass_utils import run_bass_kernel_spmd

P = 128
N = 100000
NCORES = 8
NPC = N // NCORES            # 12500 nodes per core
NBLK = (NPC + P - 1) // P    # 98 dst blocks per core (last has 84)
CH = 4                       # src chunks (int16 gather index limit)
QROWS = NPC // CH            # 3125: per-rank quarter contributed to a chunk
CHROWS = QROWS * NCORES      # 25000 rows per chunk table
GT = 48                      # tiles (of 128 edges) per dma_gather
LSTM_CHUNK = 500             # nodes per LSTM/matmul column chunk

bf16 = ml_dtypes.bfloat16
f32 = np.float32


# ---------------------------------------------------------------- host prep


def _prep_edges(edge_index):
    """Sort/pad each core's incident edges into a cross-core-uniform tile
    schedule. Returns per-core device arrays + the static schedule."""
    src = np.asarray(edge_index[0], dtype=np.int64)
    dst = np.asarray(edge_index[1], dtype=np.int64)
    loops = np.arange(N, dtype=np.int64)
    src = np.concatenate([src, loops])
    dst = np.concatenate([dst, loops])

    deg = np.bincount(dst, minlength=N).astype(np.float64)
    dinv = 1.0 / np.sqrt(deg)
    norm = (dinv[src] * dinv[dst]).astype(np.float32)

    core_of = dst // NPC
    per_core = []
    counts = np.zeros((NCORES, CH, NBLK), np.int64)
    for c in range(NCORES):
        m = core_of == c
        s = src[m]
        d = dst[m] - c * NPC
        w = norm[m]
        ch = (s % NPC) // QROWS
        o = np.lexsort((d, ch))
        s, d, w, ch = s[o], d[o], w[o], ch[o]
        b = d // P
        counts[c] = np.bincount(ch * NBLK + b, minlength=CH * NBLK).reshape(
            CH, NBLK
        )
        per_core.append((s, d, w, ch, b))

    # tiles per (chunk, block) run: padded to the max across cores
    T_run = (counts.max(axis=0) + P - 1) // P          # [CH, NBLK]
    flat = T_run.reshape(-1)
    base = np.zeros(CH * NBLK + 1, np.int64)
    np.cumsum(flat, out=base[1:])                      # tile offset per run
    TT = int(base[-1])
    NIDX = TT * P
    ctb = [int(base[ch * NBLK]) for ch in range(CH)] + [TT]  # chunk tile base

    arrs = []
    for c in range(NCORES):
        s, d, w, ch, b = per_core[c]
        gid = ch * NBLK + b
        cnt = counts[c].reshape(-1)
        gstart = np.concatenate([[0], np.cumsum(cnt)[:-1]])
        within = np.arange(len(s)) - gstart[gid]
        pos = base[gid] * P + within

        idxs = np.zeros(NIDX, np.int16)                 # pad -> row 0 (valid)
        # chunk q table = concat over ranks of each rank's q-th quarter
        idxs[pos] = ((s // NPC) * QROWS + (s % QROWS)).astype(np.int16)
        dstv = np.full(NIDX, -1.0, np.float32)          # pad -> no dst match
        dstv[pos] = (d - b * P).astype(np.float32)
        nrmv = np.zeros(NIDX, np.float32)
        nrmv[pos] = w

        idx16 = np.tile(np.ascontiguousarray(idxs.reshape(-1, 16).T), (8, 1))
        dstt = np.ascontiguousarray(dstv.reshape(TT, P).T)
        nrmt = np.ascontiguousarray(nrmv.reshape(TT, P).T)
        arrs.append((idx16, dstt, nrmt))

    # gather pieces: per chunk, consecutive groups of <= GT tiles
    pieces = []
    for chn in range(CH):
        t0, t1 = ctb[chn], ctb[chn + 1]
        pieces.append([(t, min(GT, t1 - t)) for t in range(t0, t1, GT)])

    sched = dict(T_run=T_run, base=base, TT=TT, NIDX=NIDX, ctb=ctb, pieces=pieces)
    return arrs, sched


# ---------------------------------------------------------------- device


def _build_nc(sched):
    T_run, base, TT, NIDX, ctb, pieces = (
        sched["T_run"],
        sched["base"],
        sched["TT"],
        sched["NIDX"],
        sched["ctb"],
        sched["pieces"],
    )
    dt = mybir.dt
    alu = mybir.AluOpType
    act = mybir.ActivationFunctionType

    nc = bacc.Bacc("TRN2", target_bir_lowering=False, debug=False, num_devices=NCORES)

    # ---- I/O
    zT_d = nc.dram_tensor("zT", [P, NPC], dt.bfloat16, kind="ExternalInput")
    idx_d = nc.dram_tensor("idx16", [P, NIDX // 16], dt.int16, kind="ExternalInput")
    dst_d = nc.dram_tensor("dstv", [P, TT], dt.float32, kind="ExternalInput")
    nrm_d = nc.dram_tensor("nrmv", [P, TT], dt.float32, kind="ExternalInput")
    iota_d = nc.dram_tensor("iota", [P, P], dt.bfloat16, kind="ExternalInput")
    wih_d = {
        g: nc.dram_tensor(f"wih_{g}", [P, P], dt.bfloat16, kind="ExternalInput")
        for g in "igo"
    }
    bg_d = {
        g: nc.dram_tensor(f"bg_{g}", [P, 1], dt.float32, kind="ExternalInput")
        for g in "igo"
    }
    w1_d = nc.dram_tensor("w1", [P, P], dt.bfloat16, kind="ExternalInput")
    w2_d = nc.dram_tensor("w2", [P, P], dt.bfloat16, kind="ExternalInput")
    w3t_d = nc.dram_tensor("w3t", [P, P], dt.bfloat16, kind="ExternalInput")
    b1_d = nc.dram_tensor("b1", [P, 1], dt.float32, kind="ExternalInput")
    b2_d = nc.dram_tensor("b2", [P, 1], dt.float32, kind="ExternalInput")
    b3_d = nc.dram_tensor("b3", [P, 1], dt.float32, kind="ExternalInput")
    out_d = nc.dram_tensor("outT", [P, NPC], dt.float32, kind="ExternalOutput")

    bounce = [nc.dram_tensor(f"bounce{l}", [NPC, P], dt.bfloat16) for l in range(2)]
    table = [
        [nc.dram_tensor(f"table{l}_{q}", [CHROWS, P], dt.bfloat16) for q in range(CH)]
        for l in range(2)
    ]

    with tile.TileContext(nc) as tc, ExitStack() as ctx:
        konst = ctx.enter_context(tc.tile_pool(name="konst", bufs=1))
        big = ctx.enter_context(tc.tile_pool(name="big", bufs=1))

        def load_const(handle, shape, dtype):
            t = konst.tile(shape, dtype, tag=handle.name)
            nc.sync.dma_start(t[:], handle[:])
            return t

        iota_t = load_const(iota_d, [P, P], dt.bfloat16)
        wih_t = {g: load_const(wih_d[g], [P, P], dt.bfloat16) for g in "igo"}
        bg_t = {g: load_const(bg_d[g], [P, 1], dt.float32) for g in "igo"}
        w1_t = load_const(w1_d, [P, P], dt.bfloat16)
        w2_t = load_const(w2_d, [P, P], dt.bfloat16)
        w3t_t = load_const(w3t_d, [P, P], dt.bfloat16)
        b1_t = load_const(b1_d, [P, 1], dt.float32)
        b2_t = load_const(b2_d, [P, 1], dt.float32)
        b3_t = load_const(b3_d, [P, 1], dt.float32)
        idx_t = load_const(idx_d, [P, NIDX // 16], dt.int16)
        dst_t = load_const(dst_d, [P, TT], dt.float32)
        nrm_t = load_const(nrm_d, [P, TT], dt.float32)

        xT_t = big.tile([P, NPC], dt.bfloat16, tag="xT")  # x1T then x2T

        # ---------------- phase 1: LSTM -> hT (feature-major, bf16)
        with tc.tile_pool(name="h_pool", bufs=1) as hpool:
            hT_t = hpool.tile([P, NPC], dt.bfloat16, tag="hT")
            with (
                tc.tile_pool(name="lstm_sb", bufs=1) as lsb,
                tc.tile_pool(name="lstm_ps", bufs=6, space="PSUM") as lps,
                tc.tile_pool(name="lstm_tr", bufs=8) as ltr,
            ):
                zT_t = lsb.tile([P, NPC], dt.bfloat16, tag="zT")
                nc.sync.dma_start(zT_t[:], zT_d[:])

                nchunk = (NPC + LSTM_CHUNK - 1) // LSTM_CHUNK
                for k in range(nchunk):
                    c0 = k * LSTM_CHUNK
                    c1 = min(NPC, c0 + LSTM_CHUNK)
                    w = c1 - c0
                    gate = {}
                    for g in "igo":
                        ps = lps.tile([P, LSTM_CHUNK], dt.float32, tag="ps")
                        nc.tensor.matmul(
                            ps[:, :w], wih_t[g][:], zT_t[:, c0:c1], start=True, stop=True
                        )
                        fn = act.Tanh if g == "g" else act.Sigmoid
                        sg = ltr.tile([P, LSTM_CHUNK], dt.bfloat16, tag="sg" + g)
                        nc.scalar.activation(sg[:, :w], ps[:, :w], fn, bias=bg_t[g][:])
                        gate[g] = sg
                    ct = ltr.tile([P, LSTM_CHUNK], dt.bfloat16, tag="ct")
                    nc.vector.tensor_tensor(
                        ct[:, :w], gate["i"][:, :w], gate["g"][:, :w], op=alu.mult
                    )
                    th = ltr.tile([P, LSTM_CHUNK], dt.bfloat16, tag="th")
                    nc.scalar.activation(th[:, :w], ct[:, :w], act.Tanh)
                    nc.vector.tensor_tensor(
                        hT_t[:, c0:c1], gate["o"][:, :w], th[:, :w], op=alu.mult
                    )

            # ---------------- phase 2: m1 = (h @ W1) node-major -> bounce0
            _mm_to_bounce(nc, tc, hT_t, w1_t, bounce[0])

        _allgather(nc, bounce[0], table[0])

        with (
            tc.tile_pool(name="stag", bufs=6) as stag,
            tc.tile_pool(name="spool", bufs=12) as spool,
        ):
            # ------------- phase 3: edge layer 1 -> x1T = relu(agg + b1)
            def post1(b, nb, pa):
                nc.scalar.activation(
                    xT_t[:, b * P : b * P + nb], pa[:, :nb], act.Relu, bias=b1_t[:]
                )

            _edge_phase(nc, tc, table[0], sched, idx_t, dst_t, nrm_t, iota_t, stag, spool, post1)

            # ------------- phase 4: m2 = (x1 @ W2) node-major -> bounce1
            _mm_to_bounce(nc, tc, xT_t, w2_t, bounce[1])
            _allgather(nc, bounce[1], table[1])

            # ------------- phase 5: edge layer 2 -> x2T = agg + b2 (no relu)
            def post2(b, nb, pa):
                nc.vector.tensor_scalar(
                    xT_t[:, b * P : b * P + nb], pa[:, :nb], b2_t[:], None, op0=alu.add
                )

            _edge_phase(nc, tc, table[1], sched, idx_t, dst_t, nrm_t, iota_t, stag, spool, post2)

        # ---------------- phase 6: outT = relu(W3T.T @ x2T + b3)
        with (
            tc.tile_pool(name="out_ps", bufs=3, space="PSUM") as ops,
            tc.tile_pool(name="out_sb", bufs=3) as osb,
        ):
            nchunk = (NPC + LSTM_CHUNK - 1) // LSTM_CHUNK
            for k in range(nchunk):
                c0 = k * LSTM_CHUNK
                c1 = min(NPC, c0 + LSTM_CHUNK)
                w = c1 - c0
                ps = ops.tile([P, LSTM_CHUNK], dt.float32, tag="ps")
                nc.tensor.matmul(
                    ps[:, :w], w3t_t[:], xT_t[:, c0:c1], start=True, stop=True
                )
                ot = osb.tile([P, LSTM_CHUNK], dt.float32, tag="ot")
                nc.scalar.activation(ot[:, :w], ps[:, :w], act.Relu, bias=b3_t[:])
                nc.sync.dma_start(out_d[:, c0:c1], ot[:, :w])

    nc.compile()
    return nc


def _mm_to_bounce(nc, tc, featT, w_t, bounce_d):
    """Per 128-node block: matmul(lhsT=featT block, rhs=W) -> node-major
    [node, feat] psum -> bf16 stage -> one strided DMA into bounce DRAM."""
    dt = mybir.dt
    act = mybir.ActivationFunctionType
    with (
        tc.tile_pool(name="m_ps", bufs=2, space="PSUM") as mps,
        tc.tile_pool(name="m_sb", bufs=1) as msb,
    ):
        stage = msb.tile([P, NBLK * P], dt.bfloat16, tag="mstage")
        for b in range(NBLK):
            nb = min(P, NPC - b * P)
            pm = mps.tile([P, P], dt.float32, tag="pm")
            nc.tensor.matmul(
                pm[:nb, :], featT[:, b * P : b * P + nb], w_t[:], start=True, stop=True
            )
            nc.scalar.activation(
                stage[:nb, b * P : (b + 1) * P], pm[:nb, :], act.Copy
            )
        full = (NPC // P) * P  # 12416
        nc.sync.dma_start(
            bounce_d[:full, :].rearrange("(b p) f -> p b f", p=P),
            stage[:, : NPC // P * P].rearrange("p (b f) -> p b f", f=P),
        )
        rem = NPC - full
        if rem:
            nc.sync.dma_start(bounce_d[full:, :], stage[:rem, full:])


def _allgather(nc, bounce_d, tables_d):
    # one sub-AllGather per quarter: output q IS chunk table q (offset-free),
    # and chunk-q edge gathers can start as soon as AG#q lands.
    for q in range(CH):
        nc.gpsimd.collective_compute(
            "AllGather",
            mybir.AluOpType.bypass,
            replica_groups=[list(range(NCORES))],
            ins=[bounce_d[q * QROWS : (q + 1) * QROWS, :]],
            outs=[tables_d[q][:]],
        )


def _edge_phase(nc, tc, table_d, sched, idx_t, dst_t, nrm_t, iota_t, stag, spool, post):
    dt = mybir.dt
    alu = mybir.AluOpType
    T_run, base, ctb, pieces = (
        sched["T_run"],
        sched["base"],
        sched["ctb"],
        sched["pieces"],
    )
    piece_tiles = {}
    with tc.tile_pool(name="agg_ps", bufs=6, space="PSUM") as aps:
        for b in range(NBLK):
            nb = min(P, NPC - b * P)
            pa = aps.tile([P, P], dt.float32, tag="pa")
            ntile_b = int(T_run[:, b].sum())
            done = 0
            for chn in range(CH):
                for t in range(int(T_run[chn][b])):
                    gt = int(base[chn * NBLK + b]) + t
                    rel = gt - ctb[chn]
                    pi, slot = divmod(rel, GT)
                    key = (chn, pi)
                    if key not in piece_tiles:
                        pt0, pnt = pieces[chn][pi]
                        stg = stag.tile([P, GT, P], dt.bfloat16, tag="stag")
                        nc.gpsimd.dma_gather(
                            stg[:, :pnt, :],
                            table_d[chn][:],
                            idx_t[:, pt0 * 8 : (pt0 + pnt) * 8],
                            pnt * P,
                            pnt * P,
                            P,
                            single_packet=False,
                        )
                        piece_tiles[key] = stg
                    stg = piece_tiles[key]
                    st = spool.tile([P, P], dt.bfloat16, tag="st")
                    nc.vector.tensor_scalar(
                        st[:],
                        iota_t[:],
                        dst_t[:, gt : gt + 1],
                        nrm_t[:, gt : gt + 1],
                        op0=alu.is_equal,
                        op1=alu.mult,
                    )
                    nc.tensor.matmul(
                        pa[:],
                        stg[:, slot, :],
                        st[:],
                        start=(done == 0),
                        stop=(done == ntile_b - 1),
                    )
                    done += 1
            post(b, nb, pa)


# ---------------------------------------------------------------- entry


def build(z, edge_index, W_ih, W_hh, b_ih, b_hh, W1, b1, W2, b2, W3, b3):
    """Host prep + trace + compile. Returns (nc, in_maps)."""
    z = np.asarray(z, dtype=np.float32)
    W_ih = np.asarray(W_ih, dtype=np.float32)
    b = np.asarray(b_ih, dtype=np.float32) + np.asarray(b_hh, dtype=np.float32)

    arrs, sched = _prep_edges(edge_index)
    nc = _build_nc(sched)

    gi = {"i": 0, "g": 2, "o": 3}  # torch gate order i,f,g,o (f unused: c0=0)
    common = {
        "iota": np.ascontiguousarray(
            np.tile(np.arange(P, dtype=np.float32), (P, 1))
        ).astype(bf16),
        "w1": np.asarray(W1, np.float32).astype(bf16),
        "w2": np.asarray(W2, np.float32).astype(bf16),
        "w3t": np.ascontiguousarray(np.asarray(W3, np.float32).T).astype(bf16),
        "b1": np.asarray(b1, np.float32).reshape(P, 1).copy(),
        "b2": np.asarray(b2, np.float32).reshape(P, 1).copy(),
        "b3": np.asarray(b3, np.float32).reshape(P, 1).copy(),
    }
    for g, k in gi.items():
        common[f"wih_{g}"] = np.ascontiguousarray(
            W_ih[k * P : (k + 1) * P, :].T
        ).astype(bf16)
        common[f"bg_{g}"] = b[k * P : (k + 1) * P].reshape(P, 1).copy()

    in_maps = []
    for c in range(NCORES):
        idx16, dstt, nrmt = arrs[c]
        m = dict(common)
        m["zT"] = np.ascontiguousarray(z[c * NPC : (c + 1) * NPC].T).astype(bf16)
        m["idx16"] = idx16
        m["dstv"] = dstt
        m["nrmv"] = nrmt
        in_maps.append(m)
    return nc, in_maps


def assemble(results):
    out = np.empty((N, P), np.float32)
    for c in range(NCORES):
        out[c * NPC : (c + 1) * NPC] = results[c]["outT"].T
    return out


def kernel(z, edge_index, W_ih, W_hh, b_ih, b_hh, W1, b1, W2, b2, W3, b3):
    nc, in_maps = build(z, edge_index, W_ih, W_hh, b_ih, b_hh, W1, b1, W2, b2, W3, b3)
    res = run_bass_kernel_spmd(nc, in_maps, core_ids=list(range(NCORES)))
    return assemble(res.results)

